# revision 5
# baseline (speedup 1.0000x reference)
"""Trainium2 Bass kernel for nn_EquivariantMessagePassing.

Math (verified vs reference):
  - h1e/h2e branches in the reference are dead code (deleted before use).
  - The two e3nn linears fuse into 4 constant matrices M[k] (160x160) so the
    per-edge message is  msg(e) = sum_k attr_k(e) * (x[src(e)] @ M[k]).
  - The linear commutes with the scatter-add, so per node n:
      agg[n] = sum_k T_k[n] @ M[k],   T_k[n] = sum_{e->n} attr_k(e) * x[src(e)]
    T_k is computed on the tensor engine as (attr-scaled one-hot).T @ gathered_x
    with PSUM accumulation, then a small dense matmul applies M[k].

Sharding: nodes are bin-packed onto 8 cores (by edge count) -> no collectives;
each core owns its nodes' full aggregation. Within a core, nodes are packed
into 80 chunks of <=32 nodes and <=512 edges; each chunk is 4 edge-tiles of
128. The one-hot tiles are built on the host (they are input data), the node
table is gathered on-device with dma_gather.
"""

import numpy as np

import concourse.bass as bass
import concourse.bacc as bacc
import concourse.mybir as mybir
import concourse.tile as tile
from concourse import bass_utils

F32 = mybir.dt.float32
I16 = mybir.dt.int16

N_NODES = 20000
N_EDGES = 320000
D_IN = 160
ELEM = 192              # padded node row (f32) -> 768B, multiple of 256B
N_CORES = 8
CHUNK_NODES = 32        # one-hot block width (4 attr blocks * 32 = 128 = M)
CHUNKS = 80             # chunks per core
GROUPS = CHUNKS // 4    # 4 chunks = 128 output rows per group
NPC_ROWS = CHUNKS * CHUNK_NODES   # 2560 output rows per core
SQ3 = float(np.sqrt(3.0))

_CACHE = {}


def _build_M(W0, W1, V0, V1):
    """The 4 fused per-attr-component matrices M[k][f_in, g_out]."""
    U0 = (W0 @ V0) / np.sqrt(96.0 * 128.0)      # [96,64]
    U1 = (W1 @ V1) / np.sqrt(96.0 * 64.0)       # [96,32]
    U0a, U0b = U0[:64], U0[64:]
    U1a, U1b = U1[:64], U1[64:]
    M = np.zeros((4, 160, 160), np.float32)
    M[0, :64, :64] = U0a
    for i in range(3):
        M[0, 64 + i:160:3, 64 + i:160:3] = U1b
        M[1 + i, 64 + i:160:3, :64] = U0b / SQ3
        M[1 + i, :64, 64 + i:160:3] = U1a
    return M


def _pack(deg):
    """Assign each node to (core, chunk, col). Greedy balanced packing."""
    order = np.argsort(-deg, kind="stable")
    core_edges = np.zeros(N_CORES, np.int64)
    core_nodes = np.zeros(N_CORES, np.int64)
    core_of = np.empty(N_NODES, np.int32)
    max_nodes_per_core = NPC_ROWS
    for u in order:
        cands = np.nonzero(core_nodes < max_nodes_per_core)[0]
        c = cands[np.argmin(core_edges[cands])]
        core_of[u] = c
        core_edges[c] += deg[u]
        core_nodes[c] += 1
    cap = int(np.ceil(core_edges.max() / (CHUNKS * 128.0))) * 128
    cap = max(cap, 512)

    chunk_of = np.empty(N_NODES, np.int32)
    col_of = np.empty(N_NODES, np.int32)
    for c in range(N_CORES):
        nodes_c = order[core_of[order] == c]   # still degree-desc
        ce = np.zeros(CHUNKS, np.int64)
        cn = np.zeros(CHUNKS, np.int64)
        for u in nodes_c:
            k = np.nonzero(cn < CHUNK_NODES)[0]
            b = k[np.argmin(ce[k])]
            chunk_of[u] = b
            col_of[u] = cn[b]
            ce[b] += deg[u]
            cn[b] += 1
        cap = max(cap, int(np.ceil(ce.max() / 128.0)) * 128)
    return core_of, chunk_of, col_of, cap


def _build_program(tpc):
    """One SPMD program; per-core data differs. Returns (nc, names)."""
    nc = bacc.Bacc("TRN2", target_bir_lowering=False, debug=False)
    spc = CHUNKS * tpc * 128          # slots per core
    nodes_d = nc.dram_tensor("nodes", [N_NODES, ELEM], F32, kind="ExternalInput")
    idx_d = nc.dram_tensor("idx", [128, spc // 16], I16, kind="ExternalInput")
    ohot_d = nc.dram_tensor("ohot", [CHUNKS * tpc, 128, 128], F32,
                            kind="ExternalInput")
    mwa_d = nc.dram_tensor("mwa", [128, 640], F32, kind="ExternalInput")
    mwb_d = nc.dram_tensor("mwb", [32, 640], F32, kind="ExternalInput")
    ident_d = nc.dram_tensor("ident", [128, 128], F32, kind="ExternalInput")
    gam_d = nc.dram_tensor("gam", [128, 160], F32, kind="ExternalInput")
    bet_d = nc.dram_tensor("bet", [128, 160], F32, kind="ExternalInput")
    out_d = nc.dram_tensor("out", [NPC_ROWS, 160], F32, kind="ExternalOutput")

    X = mybir.AxisListType.X
    with tile.TileContext(nc) as tc:
        with (
            tc.tile_pool(name="const", bufs=1) as const,
            tc.tile_pool(name="xgp", bufs=3) as xgp,
            tc.tile_pool(name="ohp", bufs=3) as ohp,
            tc.tile_pool(name="tsbp", bufs=3) as tsbp,
            tc.tile_pool(name="grp", bufs=2) as grp,
            tc.tile_pool(name="lnp", bufs=2) as lnp,
            tc.tile_pool(name="tps", bufs=2, space="PSUM") as tps,
            tc.tile_pool(name="ttps", bufs=2, space="PSUM") as ttps,
            tc.tile_pool(name="aps", bufs=2, space="PSUM") as aps,
        ):
            eps_sb = const.tile([128, 1], F32, tag="eps")
            nc.gpsimd.memset(eps_sb[:], 1e-5)
            idx_sb = const.tile([128, spc // 16], I16, tag="idx")
            nc.sync.dma_start(idx_sb[:], idx_d[:])
            mwa_sb = const.tile([128, 640], F32, tag="mwa")
            nc.sync.dma_start(mwa_sb[:], mwa_d[:])
            mwb_sb = const.tile([32, 640], F32, tag="mwb")
            nc.sync.dma_start(mwb_sb[:], mwb_d[:])
            ident_sb = const.tile([128, 128], F32, tag="ident")
            nc.sync.dma_start(ident_sb[:], ident_d[:])
            gam_sb = const.tile([128, 160], F32, tag="gam")
            nc.sync.dma_start(gam_sb[:], gam_d[:])
            bet_sb = const.tile([128, 160], F32, tag="bet")
            nc.sync.dma_start(bet_sb[:], bet_d[:])

            for g in range(GROUPS):
                ga = grp.tile([128, 512], F32, tag="ga")
                gb = grp.tile([32, 512], F32, tag="gb")
                for j in range(4):
                    c = 4 * g + j
                    xg = xgp.tile([128, tpc, ELEM], F32, tag="xg")
                    nc.gpsimd.dma_gather(
                        xg[:], nodes_d[:],
                        idx_sb[:, tpc * 8 * c:tpc * 8 * (c + 1)],
                        tpc * 128, tpc * 128, ELEM,
                    )
                    oh = ohp.tile([128, tpc, 128], F32, tag="oh")
                    nc.sync.dma_start(
                        oh[:],
                        ohot_d[c * tpc:(c + 1) * tpc].rearrange("t p f -> p t f"),
                    )
                    tch = tps.tile([128, 160], F32, tag="tch")
                    for t in range(tpc):
                        nc.tensor.matmul(
                            tch[:], oh[:, t, :], xg[:, t, 0:160],
                            start=(t == 0), stop=(t == tpc - 1),
                        )
                    tsb = tsbp.tile([128, 160], F32, tag="tsb")
                    # DVE, not ACT: ScalarE reads of multi-matmul PSUM
                    # accumulation groups fault flakily on this HW
                    nc.vector.tensor_copy(tsb[:], tch[:])
                    tta = ttps.tile([128, 128], F32, tag="tta")
                    ttb = ttps.tile([32, 128], F32, tag="ttb")
                    nc.tensor.transpose(tta[:], tsb[:, 0:128], ident_sb[:])
                    nc.tensor.transpose(ttb[:], tsb[:, 128:160], ident_sb[:])
                    # scatter the (k,n) columns into k-major layout (k, j, n)
                    ga4 = ga[:].rearrange("p (q j n) -> p q j n", q=4, j=4)
                    gb4 = gb[:].rearrange("p (q j n) -> p q j n", q=4, j=4)
                    tta3 = tta[:].rearrange("p (q n) -> p q n", q=4)
                    ttb3 = ttb[:].rearrange("p (q n) -> p q n", q=4)
                    nc.vector.tensor_copy(ga4[:, :, j, :], tta3)
                    nc.scalar.copy(gb4[:, :, j, :], ttb3)
                agg = aps.tile([128, 160], F32, tag="agg")
                for k in range(4):
                    nc.tensor.matmul(
                        agg[:], ga[:, 128 * k:128 * (k + 1)],
                        mwa_sb[:, 160 * k:160 * (k + 1)],
                        start=(k == 0), stop=False,
                    )
                    nc.tensor.matmul(
                        agg[:], gb[:, 128 * k:128 * (k + 1)],
                        mwb_sb[:, 160 * k:160 * (k + 1)],
                        start=False, stop=(k == 3),
                    )
                # LayerNorm over the 160 features (free dim)
                asb = lnp.tile([128, 160], F32, tag="asb")
                nc.vector.tensor_copy(asb[:], agg[:])
                sums = lnp.tile([128, 1], F32, tag="s1")
                nc.vector.reduce_sum(sums[:], agg[:], axis=X)
                mu = lnp.tile([128, 1], F32, tag="s2")
                nc.vector.tensor_scalar_mul(mu[:], sums[:], 1.0 / 160.0)
                sq = lnp.tile([128, 160], F32, tag="sq")
                nc.scalar.square(sq[:], asb[:])
                sqs = lnp.tile([128, 1], F32, tag="s3")
                nc.vector.reduce_sum(sqs[:], sq[:], axis=X)
                musq = lnp.tile([128, 1], F32, tag="s4")
                nc.vector.tensor_mul(musq[:], mu[:], mu[:])
                var = lnp.tile([128, 1], F32, tag="s5")
                nc.vector.tensor_scalar_mul(var[:], sqs[:], 1.0 / 160.0)
                var2 = lnp.tile([128, 1], F32, tag="s6")
                nc.vector.tensor_sub(var2[:], var[:], musq[:])
                std = lnp.tile([128, 1], F32, tag="s7")
                nc.scalar.activation(std[:], var2[:],
                                     mybir.ActivationFunctionType.Sqrt,
                                     bias=eps_sb[:])
                rstd = lnp.tile([128, 1], F32, tag="s8")
                nc.vector.reciprocal(rstd[:], std[:])
                nrm = lnp.tile([128, 160], F32, tag="nrm")
                nc.vector.tensor_scalar(
                    nrm[:], asb[:], mu[:], rstd[:],
                    op0=mybir.AluOpType.subtract, op1=mybir.AluOpType.mult,
                )
                nrm2 = lnp.tile([128, 160], F32, tag="nrm2")
                nc.vector.tensor_mul(nrm2[:], nrm[:], gam_sb[:])
                osb = lnp.tile([128, 160], F32, tag="osb")
                nc.vector.tensor_add(osb[:], nrm2[:], bet_sb[:])
                nc.sync.dma_start(out_d[128 * g:128 * (g + 1), :], osb[:])

    nc.compile()
    return nc


def _prepare(node_features, edge_index, edge_attr, W0, W1, V0, V1,
             gamma, beta):
    src = edge_index[0].astype(np.int64)
    dst = edge_index[1].astype(np.int64)
    deg = np.bincount(dst, minlength=N_NODES)

    core_of, chunk_of, col_of, cap = _pack(deg)
    tpc = cap // 128
    spc = CHUNKS * tpc * 128

    # edge -> slot
    gchunk = core_of[dst] * CHUNKS + chunk_of[dst]       # global chunk id
    order = np.argsort(gchunk, kind="stable")
    gsort = gchunk[order]
    starts = np.searchsorted(gsort, np.arange(N_CORES * CHUNKS))
    within = np.arange(N_EDGES) - starts[gsort]
    slot = gsort * cap + within                          # global slot id
    assert within.max() < cap

    e_src = src[order]
    e_attr = np.asarray(edge_attr, np.float32)[order]
    e_col = col_of[dst[order]]

    # per-slot tables
    n_slots = N_CORES * CHUNKS * cap
    s_src = np.zeros(n_slots, np.int16)
    s_src[slot] = e_src.astype(np.int16)

    ohot = np.zeros((N_CORES * CHUNKS * tpc, 128, 128), np.float32)
    tile_id = slot // 128
    row_id = slot % 128
    for k in range(4):
        ohot[tile_id, row_id, 32 * k + e_col] = e_attr[:, k]

    # idx arrays: per core [128, spc//16], slot i at [i%16, i//16], tiled x8
    s_src_c = s_src.reshape(N_CORES, spc)
    idx16 = np.zeros((N_CORES, 16, spc // 16), np.int16)
    ii = np.arange(spc)
    idx16[:, ii % 16, ii // 16] = s_src_c
    idx128 = np.tile(idx16, (1, 8, 1))

    nodes_pad = np.zeros((N_NODES, ELEM), np.float32)
    nodes_pad[:, :160] = np.asarray(node_features, np.float32)

    M = _build_M(np.asarray(W0, np.float64), np.asarray(W1, np.float64),
                 np.asarray(V0, np.float64), np.asarray(V1, np.float64))
    mwa = np.zeros((128, 640), np.float32)
    mwb = np.zeros((32, 640), np.float32)
    for k in range(4):
        mwa[:, 160 * k:160 * (k + 1)] = M[k, 0:128, :]
        mwb[:, 160 * k:160 * (k + 1)] = M[k, 128:160, :]

    gam_t = np.broadcast_to(np.asarray(gamma, np.float32), (128, 160)).copy()
    bet_t = np.broadcast_to(np.asarray(beta, np.float32), (128, 160)).copy()
    ident = np.eye(128, dtype=np.float32)

    in_maps = []
    for c in range(N_CORES):
        in_maps.append(dict(
            nodes=nodes_pad,
            idx=idx128[c],
            ohot=ohot[c * CHUNKS * tpc:(c + 1) * CHUNKS * tpc],
            mwa=mwa, mwb=mwb, ident=ident, gam=gam_t, bet=bet_t,
        ))

    # output row -> node mapping
    rows_node = np.full((N_CORES, NPC_ROWS), -1, np.int64)
    allnodes = np.arange(N_NODES)
    rows_node[core_of, 32 * chunk_of[allnodes] + col_of[allnodes]] = allnodes
    return in_maps, rows_node, tpc


def kernel(node_features, edge_index, edge_attr, node_pos,
           W0, W1, W2, W3, V0, V1, gamma, beta, _trace=False):
    node_features = np.asarray(node_features, np.float32)
    edge_index = np.asarray(edge_index)
    edge_attr = np.asarray(edge_attr, np.float32)

    in_maps, rows_node, tpc = _prepare(
        node_features, edge_index, edge_attr,
        np.asarray(W0), np.asarray(W1), np.asarray(V0), np.asarray(V1),
        np.asarray(gamma), np.asarray(beta))

    if tpc not in _CACHE:
        _CACHE[tpc] = _build_program(tpc)
    nc = _CACHE[tpc]

    res = bass_utils.run_bass_kernel_spmd(
        nc, in_maps, core_ids=list(range(N_CORES)), trace=_trace)

    full = np.zeros((N_NODES, 160), np.float32)
    for c in range(N_CORES):
        rows = rows_node[c]
        valid = rows >= 0
        full[rows[valid]] = res.results[c]["out"][valid]
    if _trace:
        kernel._last_trace = res
    return full


# revision 7
# speedup vs baseline: 1.0485x; 1.0485x over previous
"""Trainium2 Bass kernel for nn_EquivariantMessagePassing.

Math (verified vs reference):
  - h1e/h2e branches in the reference are dead code (deleted before use).
  - The two e3nn linears fuse into 4 constant matrices M[k] (160x160) so the
    per-edge message is  msg(e) = sum_k attr_k(e) * (x[src(e)] @ M[k]).
  - The linear commutes with the scatter-add, so per node n:
      agg[n] = sum_k T_k[n] @ M[k],   T_k[n] = sum_{e->n} attr_k(e) * x[src(e)]
    T_k is computed on the tensor engine as (attr-scaled one-hot).T @ gathered_x
    with PSUM accumulation, then a small dense matmul applies M[k].

Sharding: nodes are bin-packed onto 8 cores (by edge count) -> no collectives;
each core owns its nodes' full aggregation. Within a core, nodes are packed
into 80 chunks of <=32 nodes and <=512 edges; each chunk is 4 edge-tiles of
128. The one-hot tiles are built on the host (they are input data), the node
table is gathered on-device with dma_gather.
"""

import numpy as np

import concourse.bass as bass
import concourse.bacc as bacc
import concourse.mybir as mybir
import concourse.tile as tile
from concourse import bass_utils

F32 = mybir.dt.float32
I16 = mybir.dt.int16

N_NODES = 20000
N_EDGES = 320000
D_IN = 160
ELEM = 192              # padded node row (f32) -> 768B, multiple of 256B
N_CORES = 8
CHUNK_NODES = 32        # one-hot block width (4 attr blocks * 32 = 128 = M)
CHUNKS = 80             # chunks per core
GROUPS = CHUNKS // 4    # 4 chunks = 128 output rows per group
NPC_ROWS = CHUNKS * CHUNK_NODES   # 2560 output rows per core
SQ3 = float(np.sqrt(3.0))

_CACHE = {}


def _build_M(W0, W1, V0, V1):
    """The 4 fused per-attr-component matrices M[k][f_in, g_out]."""
    U0 = (W0 @ V0) / np.sqrt(96.0 * 128.0)      # [96,64]
    U1 = (W1 @ V1) / np.sqrt(96.0 * 64.0)       # [96,32]
    U0a, U0b = U0[:64], U0[64:]
    U1a, U1b = U1[:64], U1[64:]
    M = np.zeros((4, 160, 160), np.float32)
    M[0, :64, :64] = U0a
    for i in range(3):
        M[0, 64 + i:160:3, 64 + i:160:3] = U1b
        M[1 + i, 64 + i:160:3, :64] = U0b / SQ3
        M[1 + i, :64, 64 + i:160:3] = U1a
    return M


def _pack(deg):
    """Assign each node to (core, chunk, col). Greedy balanced packing."""
    order = np.argsort(-deg, kind="stable")
    core_edges = np.zeros(N_CORES, np.int64)
    core_nodes = np.zeros(N_CORES, np.int64)
    core_of = np.empty(N_NODES, np.int32)
    max_nodes_per_core = NPC_ROWS
    for u in order:
        cands = np.nonzero(core_nodes < max_nodes_per_core)[0]
        c = cands[np.argmin(core_edges[cands])]
        core_of[u] = c
        core_edges[c] += deg[u]
        core_nodes[c] += 1
    cap = int(np.ceil(core_edges.max() / (CHUNKS * 128.0))) * 128
    cap = max(cap, 512)

    chunk_of = np.empty(N_NODES, np.int32)
    col_of = np.empty(N_NODES, np.int32)
    for c in range(N_CORES):
        nodes_c = order[core_of[order] == c]   # still degree-desc
        ce = np.zeros(CHUNKS, np.int64)
        cn = np.zeros(CHUNKS, np.int64)
        for u in nodes_c:
            k = np.nonzero(cn < CHUNK_NODES)[0]
            b = k[np.argmin(ce[k])]
            chunk_of[u] = b
            col_of[u] = cn[b]
            ce[b] += deg[u]
            cn[b] += 1
        cap = max(cap, int(np.ceil(ce.max() / 128.0)) * 128)
    return core_of, chunk_of, col_of, cap


def _build_program(tpc):
    """One SPMD program; per-core data differs. Returns (nc, names)."""
    nc = bacc.Bacc("TRN2", target_bir_lowering=False, debug=False,
                   num_swdge_queues=4)
    spc = CHUNKS * tpc * 128          # slots per core
    nodes_d = nc.dram_tensor("nodes", [N_NODES, ELEM], F32, kind="ExternalInput")
    idx_d = nc.dram_tensor("idx", [128, spc // 16], I16, kind="ExternalInput")
    ohot_d = nc.dram_tensor("ohot", [CHUNKS * tpc, 128, 128], F32,
                            kind="ExternalInput")
    mwa_d = nc.dram_tensor("mwa", [128, 640], F32, kind="ExternalInput")
    mwb_d = nc.dram_tensor("mwb", [32, 640], F32, kind="ExternalInput")
    ident_d = nc.dram_tensor("ident", [128, 128], F32, kind="ExternalInput")
    gam_d = nc.dram_tensor("gam", [128, 160], F32, kind="ExternalInput")
    bet_d = nc.dram_tensor("bet", [128, 160], F32, kind="ExternalInput")
    out_d = nc.dram_tensor("out", [NPC_ROWS, 160], F32, kind="ExternalOutput")

    X = mybir.AxisListType.X
    with tile.TileContext(nc) as tc:
        with (
            tc.tile_pool(name="const", bufs=1) as const,
            tc.tile_pool(name="xgp", bufs=3) as xgp,
            tc.tile_pool(name="ohp", bufs=3) as ohp,
            tc.tile_pool(name="tsbp", bufs=3) as tsbp,
            tc.tile_pool(name="grp", bufs=2) as grp,
            tc.tile_pool(name="lnp", bufs=2) as lnp,
            tc.tile_pool(name="tps", bufs=2, space="PSUM") as tps,
            tc.tile_pool(name="ttps", bufs=2, space="PSUM") as ttps,
            tc.tile_pool(name="aps", bufs=2, space="PSUM") as aps,
        ):
            eps_sb = const.tile([128, 1], F32, tag="eps")
            nc.gpsimd.memset(eps_sb[:], 1e-5)
            idx_sb = const.tile([128, spc // 16], I16, tag="idx")
            nc.sync.dma_start(idx_sb[:], idx_d[:])
            mwa_sb = const.tile([128, 640], F32, tag="mwa")
            nc.sync.dma_start(mwa_sb[:], mwa_d[:])
            mwb_sb = const.tile([32, 640], F32, tag="mwb")
            nc.sync.dma_start(mwb_sb[:], mwb_d[:])
            ident_sb = const.tile([128, 128], F32, tag="ident")
            nc.sync.dma_start(ident_sb[:], ident_d[:])
            gam_sb = const.tile([128, 160], F32, tag="gam")
            nc.sync.dma_start(gam_sb[:], gam_d[:])
            bet_sb = const.tile([128, 160], F32, tag="bet")
            nc.sync.dma_start(bet_sb[:], bet_d[:])

            for g in range(GROUPS):
                ga = grp.tile([128, 512], F32, tag="ga")
                gb = grp.tile([32, 512], F32, tag="gb")
                for j in range(4):
                    c = 4 * g + j
                    xg = xgp.tile([128, tpc, ELEM], F32, tag="xg")
                    nc.gpsimd.dma_gather(
                        xg[:], nodes_d[:],
                        idx_sb[:, tpc * 8 * c:tpc * 8 * (c + 1)],
                        tpc * 128, tpc * 128, ELEM,
                        queue_num=c % 4,
                    )
                    oh = ohp.tile([128, tpc, 128], F32, tag="oh")
                    nc.sync.dma_start(
                        oh[:],
                        ohot_d[c * tpc:(c + 1) * tpc].rearrange("t p f -> p t f"),
                    )
                    tch = tps.tile([128, 160], F32, tag="tch")
                    for t in range(tpc):
                        nc.tensor.matmul(
                            tch[:], oh[:, t, :], xg[:, t, 0:160],
                            start=(t == 0), stop=(t == tpc - 1),
                        )
                    tsb = tsbp.tile([128, 160], F32, tag="tsb")
                    # DVE, not ACT: ScalarE reads of multi-matmul PSUM
                    # accumulation groups fault flakily on this HW
                    nc.vector.tensor_copy(tsb[:], tch[:])
                    tta = ttps.tile([128, 128], F32, tag="tta")
                    ttb = ttps.tile([32, 128], F32, tag="ttb")
                    nc.tensor.transpose(tta[:], tsb[:, 0:128], ident_sb[:])
                    nc.tensor.transpose(ttb[:], tsb[:, 128:160], ident_sb[:])
                    # scatter the (k,n) columns into k-major layout (k, j, n)
                    ga4 = ga[:].rearrange("p (q j n) -> p q j n", q=4, j=4)
                    gb4 = gb[:].rearrange("p (q j n) -> p q j n", q=4, j=4)
                    tta3 = tta[:].rearrange("p (q n) -> p q n", q=4)
                    ttb3 = ttb[:].rearrange("p (q n) -> p q n", q=4)
                    nc.vector.tensor_copy(ga4[:, :, j, :], tta3)
                    nc.scalar.copy(gb4[:, :, j, :], ttb3)
                agg = aps.tile([128, 160], F32, tag="agg")
                for k in range(4):
                    nc.tensor.matmul(
                        agg[:], ga[:, 128 * k:128 * (k + 1)],
                        mwa_sb[:, 160 * k:160 * (k + 1)],
                        start=(k == 0), stop=False,
                    )
                    nc.tensor.matmul(
                        agg[:], gb[:, 128 * k:128 * (k + 1)],
                        mwb_sb[:, 160 * k:160 * (k + 1)],
                        start=False, stop=(k == 3),
                    )
                # LayerNorm over the 160 features (free dim)
                asb = lnp.tile([128, 160], F32, tag="asb")
                nc.vector.tensor_copy(asb[:], agg[:])
                sums = lnp.tile([128, 1], F32, tag="s1")
                nc.vector.reduce_sum(sums[:], agg[:], axis=X)
                mu = lnp.tile([128, 1], F32, tag="s2")
                nc.vector.tensor_scalar_mul(mu[:], sums[:], 1.0 / 160.0)
                sq = lnp.tile([128, 160], F32, tag="sq")
                nc.scalar.square(sq[:], asb[:])
                sqs = lnp.tile([128, 1], F32, tag="s3")
                nc.vector.reduce_sum(sqs[:], sq[:], axis=X)
                musq = lnp.tile([128, 1], F32, tag="s4")
                nc.vector.tensor_mul(musq[:], mu[:], mu[:])
                var = lnp.tile([128, 1], F32, tag="s5")
                nc.vector.tensor_scalar_mul(var[:], sqs[:], 1.0 / 160.0)
                var2 = lnp.tile([128, 1], F32, tag="s6")
                nc.vector.tensor_sub(var2[:], var[:], musq[:])
                std = lnp.tile([128, 1], F32, tag="s7")
                nc.scalar.activation(std[:], var2[:],
                                     mybir.ActivationFunctionType.Sqrt,
                                     bias=eps_sb[:])
                rstd = lnp.tile([128, 1], F32, tag="s8")
                nc.vector.reciprocal(rstd[:], std[:])
                nrm = lnp.tile([128, 160], F32, tag="nrm")
                nc.vector.tensor_scalar(
                    nrm[:], asb[:], mu[:], rstd[:],
                    op0=mybir.AluOpType.subtract, op1=mybir.AluOpType.mult,
                )
                nrm2 = lnp.tile([128, 160], F32, tag="nrm2")
                nc.vector.tensor_mul(nrm2[:], nrm[:], gam_sb[:])
                osb = lnp.tile([128, 160], F32, tag="osb")
                nc.vector.tensor_add(osb[:], nrm2[:], bet_sb[:])
                nc.sync.dma_start(out_d[128 * g:128 * (g + 1), :], osb[:])

    nc.compile()
    return nc


def _prepare(node_features, edge_index, edge_attr, W0, W1, V0, V1,
             gamma, beta):
    src = edge_index[0].astype(np.int64)
    dst = edge_index[1].astype(np.int64)
    deg = np.bincount(dst, minlength=N_NODES)

    core_of, chunk_of, col_of, cap = _pack(deg)
    tpc = cap // 128
    spc = CHUNKS * tpc * 128

    # edge -> slot
    gchunk = core_of[dst] * CHUNKS + chunk_of[dst]       # global chunk id
    order = np.argsort(gchunk, kind="stable")
    gsort = gchunk[order]
    starts = np.searchsorted(gsort, np.arange(N_CORES * CHUNKS))
    within = np.arange(N_EDGES) - starts[gsort]
    slot = gsort * cap + within                          # global slot id
    assert within.max() < cap

    e_src = src[order]
    e_attr = np.asarray(edge_attr, np.float32)[order]
    e_col = col_of[dst[order]]

    # per-slot tables
    n_slots = N_CORES * CHUNKS * cap
    s_src = np.zeros(n_slots, np.int16)
    s_src[slot] = e_src.astype(np.int16)

    ohot = np.zeros((N_CORES * CHUNKS * tpc, 128, 128), np.float32)
    tile_id = slot // 128
    row_id = slot % 128
    for k in range(4):
        ohot[tile_id, row_id, 32 * k + e_col] = e_attr[:, k]

    # idx arrays: per core [128, spc//16], slot i at [i%16, i//16], tiled x8
    s_src_c = s_src.reshape(N_CORES, spc)
    idx16 = np.zeros((N_CORES, 16, spc // 16), np.int16)
    ii = np.arange(spc)
    idx16[:, ii % 16, ii // 16] = s_src_c
    idx128 = np.tile(idx16, (1, 8, 1))

    nodes_pad = np.zeros((N_NODES, ELEM), np.float32)
    nodes_pad[:, :160] = np.asarray(node_features, np.float32)

    M = _build_M(np.asarray(W0, np.float64), np.asarray(W1, np.float64),
                 np.asarray(V0, np.float64), np.asarray(V1, np.float64))
    mwa = np.zeros((128, 640), np.float32)
    mwb = np.zeros((32, 640), np.float32)
    for k in range(4):
        mwa[:, 160 * k:160 * (k + 1)] = M[k, 0:128, :]
        mwb[:, 160 * k:160 * (k + 1)] = M[k, 128:160, :]

    gam_t = np.broadcast_to(np.asarray(gamma, np.float32), (128, 160)).copy()
    bet_t = np.broadcast_to(np.asarray(beta, np.float32), (128, 160)).copy()
    ident = np.eye(128, dtype=np.float32)

    in_maps = []
    for c in range(N_CORES):
        in_maps.append(dict(
            nodes=nodes_pad,
            idx=idx128[c],
            ohot=ohot[c * CHUNKS * tpc:(c + 1) * CHUNKS * tpc],
            mwa=mwa, mwb=mwb, ident=ident, gam=gam_t, bet=bet_t,
        ))

    # output row -> node mapping
    rows_node = np.full((N_CORES, NPC_ROWS), -1, np.int64)
    allnodes = np.arange(N_NODES)
    rows_node[core_of, 32 * chunk_of[allnodes] + col_of[allnodes]] = allnodes
    return in_maps, rows_node, tpc


def kernel(node_features, edge_index, edge_attr, node_pos,
           W0, W1, W2, W3, V0, V1, gamma, beta, _trace=False):
    node_features = np.asarray(node_features, np.float32)
    edge_index = np.asarray(edge_index)
    edge_attr = np.asarray(edge_attr, np.float32)

    in_maps, rows_node, tpc = _prepare(
        node_features, edge_index, edge_attr,
        np.asarray(W0), np.asarray(W1), np.asarray(V0), np.asarray(V1),
        np.asarray(gamma), np.asarray(beta))

    if tpc not in _CACHE:
        _CACHE[tpc] = _build_program(tpc)
    nc = _CACHE[tpc]

    res = bass_utils.run_bass_kernel_spmd(
        nc, in_maps, core_ids=list(range(N_CORES)), trace=_trace)

    full = np.zeros((N_NODES, 160), np.float32)
    for c in range(N_CORES):
        rows = rows_node[c]
        valid = rows >= 0
        full[rows[valid]] = res.results[c]["out"][valid]
    if _trace:
        kernel._last_trace = res
    return full
